# revision 18
# baseline (speedup 1.0000x reference)
"""Trainium2 Bass kernel: BEiT-style dot-product attention with relative
position bias (batch 8, seq 1025, dim 1024, 16 heads), data-parallel over
batch (one batch element per NeuronCore).

v2 design:
  - All GEMM operands bf16 (FWL weight loads, no fp32r self-load stalls).
  - Multiplicative bias: exp(s+b) = exp(s) * exp(b). ScalarE computes
    exp(s*0.125) straight from the QK psum; the host-precomputed f16
    exp(bias) factor is applied by a 2x-rate DVE multiply. No bias matmuls,
    no f32 bias adds, pad key j=1025 masked by exp(bias)=0.
  - seq padded 1025 -> 1026: 9 j-tiles x 114 rows, 3 i-blocks x 342 cols
    (no special width-1 column path).
  - Scores: row-packed QK pairs (2 heads: K=64 rows 0-63 / 64-127).
  - PV: per head stationary [114, 65] = [v | ones]; psum row 64 gives the
    softmax denominator for free.
  - Normalization: reciprocal_approx_fast on the two denominator rows,
    DRAM-roundtrip broadcast to 128 partitions, fused psum-drain multiply.
  - Software pipelining: PV of pair p-1 interleaved with QK of pair p.
  - qkv/proj biases folded into psum drains (per-partition scalar for Q/K,
    broadcast row tiles for V/proj).
"""

import os
import sys

for _p in (
    "/root/.axon_site",
    "/root/.axon_site/_ro/trn_rl_repo",
    "/root/.axon_site/_ro/pypackages",
    "/opt/trn_rl_repo",
    "/opt/pypackages",
):
    if os.path.isdir(_p) and _p not in sys.path:
        sys.path.append(_p)

import numpy as np
import ml_dtypes

import concourse.bass as bass
import concourse.bacc as bacc
import concourse.tile as tile
import concourse.mybir as mybir
from concourse.bass_utils import run_bass_kernel_spmd

F32 = mybir.dt.float32
BF16 = mybir.dt.bfloat16
F16 = mybir.dt.float16
EXPFN = mybir.ActivationFunctionType.Exp

SEQ = 1025
SP = 1026            # padded seq (9 j-tiles of 114; i padded with one zero col)
JT = 114             # j-tile rows
NJT = 9
D = 1024
H = 16
NB = 8
IBW = 342            # i-block width (3 blocks of 342 = 1026)
NIB = 3
QBLK = [(0, 512), (512, 512), (1024, 2)]

_CACHE = {}


def _build_module(vb_zero=False, pb_zero=False):
    nc = bacc.Bacc()
    xt_d = nc.dram_tensor("xt", [128, 8, SP], BF16, kind="ExternalInput")
    wq_d = nc.dram_tensor("wq", [8, 128, 8, 128], BF16, kind="ExternalInput")
    wk_d = nc.dram_tensor("wk", [8, 128, 8, 128], BF16, kind="ExternalInput")
    wv_d = nc.dram_tensor("wv", [128, 8, D], BF16, kind="ExternalInput")
    wp_d = nc.dram_tensor("wp", [128, 8, D], BF16, kind="ExternalInput")
    qb_d = nc.dram_tensor("qb", [128, 8], F32, kind="ExternalInput")
    kb_d = nc.dram_tensor("kb", [128, 8], F32, kind="ExternalInput")
    vb_d = nc.dram_tensor("vb", [1, D], BF16, kind="ExternalInput")
    pb_d = nc.dram_tensor("pb", [1, D], BF16, kind="ExternalInput")
    # exp(bias): [ib, head, p(114), jt(9), i(342)] f16, contiguous per (ib, head)
    eb_d = nc.dram_tensor("eb", [NIB, H, JT, NJT, IBW], F16, kind="ExternalInput")
    y_d = nc.dram_tensor("y", [SEQ, D], F32, kind="ExternalOutput")

    with tile.TileContext(nc) as tc:
        with (
            tc.tile_pool(name="persist", bufs=1) as pp,
            tc.tile_pool(name="consts", bufs=1) as cp,
        ):
            qt = pp.tile([128, 8, SP], BF16, tag="qt")
            kt = pp.tile([128, 8, SP], BF16, tag="kt")
            va = pp.tile([128, NJT, H, 65], F16, tag="va")
            out_all = pp.tile([128, 8, SP], BF16, tag="out_all")
            wp = pp.tile([128, 8, D], BF16, tag="wp")
            for cc2 in range(0, 8, 2):
                nc.sync.dma_start(
                    out=wp[:, cc2 : cc2 + 2, :], in_=wp_d[:, cc2 : cc2 + 2, :],
                    max_dma_last_dim=512,
                )

            qb = cp.tile([128, 8], F32, tag="qb")
            kb = cp.tile([128, 8], F32, tag="kb")
            vb = cp.tile([128, D], BF16, tag="vb")
            pb = cp.tile([128, D], BF16, tag="pb")
            nc.sync.dma_start(out=qb, in_=qb_d[:, :])
            nc.sync.dma_start(out=kb, in_=kb_d[:, :])
            nc.gpsimd.dma_start(
                out=vb,
                in_=bass.AP(tensor=vb_d, offset=0, ap=[[0, 128], [1, D]]),
            )
            nc.gpsimd.dma_start(
                out=pb,
                in_=bass.AP(tensor=pb_d, offset=0, ap=[[0, 128], [1, D]]),
            )
            onesf = cp.tile([128, NJT * H], F32, tag="onesf")
            nc.vector.memset(onesf, 1.0)
            nc.vector.tensor_copy(
                va[:, :, :, 64:65],
                onesf.rearrange("p (t h) -> p t h", t=NJT).unsqueeze(3),
            )

            # ---------------- Phase A: Q, K, V projections ----------------
            with (
                tc.tile_pool(name="xa", bufs=1) as xa,
                tc.tile_pool(name="wl", bufs=3) as wl,
                tc.tile_pool(name="psA", bufs=4, space="PSUM") as psA,
            ):
                xt = xa.tile([128, 8, SP], BF16, tag="xt")
                for ec in range(8):
                    nc.sync.dma_start(out=xt[:, ec, :], in_=xt_d[:, ec, :],
                                       max_dma_last_dim=513)

                for dst, wsrc, bias in ((qt, wq_d, qb), (kt, wk_d, kb)):
                    for ct in range(8):
                        w = wl.tile([128, 8, 128], BF16, tag="wqk")
                        nc.sync.dma_start(out=w, in_=wsrc[ct], max_dma_last_dim=512)
                        for i0, iw in QBLK:
                            pa = psA.tile([128, 512], F32, tag="psA")
                            for ec in range(8):
                                nc.tensor.matmul(
                                    pa[:, :iw],
                                    w[:, ec, :],
                                    xt[:, ec, i0 : i0 + iw],
                                    start=(ec == 0),
                                    stop=(ec == 7),
                                    skip_group_check=True,
                                )
                            nc.scalar.activation(
                                dst[:, ct, i0 : i0 + iw],
                                pa[:, :iw],
                                mybir.ActivationFunctionType.Identity,
                                bias=bias[:, ct : ct + 1],
                            )

                wv = xa.tile([128, 8, D], BF16, tag="wv")
                for ec2 in range(0, 8, 2):
                    nc.sync.dma_start(
                        out=wv[:, ec2 : ec2 + 2, :], in_=wv_d[:, ec2 : ec2 + 2, :],
                        max_dma_last_dim=512,
                    )
                for jt in range(NJT):
                    js = slice(jt * JT, (jt + 1) * JT)
                    for hb in range(2):
                        ms = slice(hb * 512, (hb + 1) * 512)
                        pa = psA.tile([128, 512], F32, tag="psA")
                        for ec in range(8):
                            nc.tensor.matmul(
                                pa[:JT, :],
                                xt[:, ec, js],
                                wv[:, ec, ms],
                                start=(ec == 0),
                                stop=(ec == 7),
                                skip_group_check=True,
                            )
                        if vb_zero:
                            nc.scalar.copy(
                                va[:JT, jt, hb * 8 : (hb + 1) * 8, 0:64],
                                pa[:JT, :].rearrange("p (h c) -> p h c", c=64),
                            )
                        else:
                            nc.vector.tensor_add(
                                va[:JT, jt, hb * 8 : (hb + 1) * 8, 0:64],
                                pa[:JT, :].rearrange("p (h c) -> p h c", c=64),
                                vb[:JT, ms].rearrange("p (h c) -> p h c", c=64),
                            )

            # ---------------- Phase B: attention ----------------
            # 3-deep software pipeline over pairs i (= ib*8+p):
            #   block i emits: eb-dma(i+..), QK(i,jt)+PV(i-1,jt)+exp(i,jt)
            #   + mult chunks(i), norm-part1(i-1) {dn,recip,cast,rd,bc},
            #   norm-part2(i-2) {muls -> out_all} + proj chunk when its ib done.
            with (
                tc.tile_pool(name="psS", bufs=2, space="PSUM") as psS,
                tc.tile_pool(name="psPV", bufs=2, space="PSUM") as psPV,
                tc.tile_pool(name="ebp", bufs=3) as ebp,
                tc.tile_pool(name="exps", bufs=2) as xp,
                tc.tile_pool(name="ep", bufs=2) as ep,
                tc.tile_pool(name="rp", bufs=3) as rp,
                tc.tile_pool(name="dramp", bufs=3, space="DRAM") as dp,
                tc.tile_pool(name="yp", bufs=2) as yp,
            ):
                def emit_proj(mt, pvt):
                    i0 = 128 * mt if mt < 8 else SEQ - 128
                    ysb = yp.tile([128, D], F32, tag="ysb")
                    for fb in range(2):
                        fs = slice(fb * 512, (fb + 1) * 512)
                        pj = pvt[:, fb, :]
                        for cc in range(8):
                            nc.tensor.matmul(
                                pj,
                                out_all[:, cc, i0 : i0 + 128],
                                wp[:, cc, fs],
                                start=(cc == 0),
                                stop=(cc == 7),
                                skip_group_check=True,
                            )
                        if pb_zero:
                            if fb == 0:
                                nc.scalar.copy(ysb[:, fs], pj)
                            else:
                                nc.vector.tensor_copy(ysb[:, fs], pj)
                        else:
                            nc.vector.tensor_add(ysb[:, fs], pj, pb[:, fs])
                    if mt < 8:
                        for yh in range(2):
                            nc.scalar.dma_start(
                                out=y_d[i0 + 64 * yh : i0 + 64 * (yh + 1), :],
                                in_=ysb[64 * yh : 64 * (yh + 1), :],
                            )
                    else:
                        nc.scalar.dma_start(
                            out=y_d[SEQ - 1 : SEQ, :], in_=ysb[127:128, :]
                        )

                def pv_acc(p, jt, pvt, e):
                    for hh in range(2):
                        nc.tensor.matmul(
                            pvt[0:65, hh, :IBW],
                            va[:JT, jt, 2 * p + hh, :],
                            e[:JT, jt, hh, :],
                            start=(jt == 0),
                            stop=(jt == NJT - 1),
                            skip_group_check=True,
                        )

                def norm_part1(st):
                    # denominators -> approx reciprocal -> bf16 -> DRAM
                    p, ib, pvt, _, _ = st
                    dn = rp.tile([1, 2, IBW], F32, tag="dn")
                    nc.vector.tensor_copy(dn, pvt[64:65, :, :IBW])
                    rr = rp.tile([1, 2, IBW], F32, tag="rr")
                    nc.vector.reciprocal_approx_fast(out=rr, in_=dn)
                    rb = rp.tile([1, 2, IBW], BF16, tag="rb")
                    nc.vector.tensor_copy(rb, rr)
                    rd = dp.tile([2, IBW], BF16, tag="rd")
                    nc.sync.dma_start(out=rd, in_=rb[0:1, :, :])
                    bc = rp.tile([128, IBW], BF16, tag="bc")
                    nc.gpsimd.dma_start(
                        out=bc,
                        in_=bass.AP(
                            tensor=rd.tensor, offset=rd.offset,
                            ap=[[IBW, 2], [0, 64], [1, IBW]],
                        ),
                    )
                    return bc

                def norm_part2(st, bc):
                    p, ib, pvt, _, _ = st
                    i0 = ib * IBW
                    tmp = rp.tile([128, IBW], F32, tag="tmp")
                    nc.vector.tensor_mul(
                        out_all[0:64, p, i0 : i0 + IBW],
                        pvt[0:64, 0, :IBW],
                        bc[0:64, :],
                    )
                    nc.vector.tensor_copy(tmp[64:128, :], pvt[0:64, 1, :IBW])
                    nc.vector.tensor_mul(
                        out_all[64:128, p, i0 : i0 + IBW],
                        tmp[64:128, :],
                        bc[64:128, :],
                    )
                    if proj_queue:
                        emit_proj(proj_queue.pop(0), pvt)
                    if p == 7:
                        proj_queue.extend(PROJ_SETS[ib])

                PROJ_SETS = {0: [0, 1], 1: [2, 3, 4], 2: [5, 6, 7, 8]}
                proj_queue = []
                s1 = None   # (p, ib, pvt, e, bc-pending)
                s2 = None
                for ib in range(NIB):
                    i0 = ib * IBW
                    for p in range(8):
                        eb0 = ebp.tile([128, NJT, IBW], F16, tag="eb0")
                        eb1 = ebp.tile([128, NJT, IBW], F16, tag="eb1")
                        for ci, j3 in enumerate(range(0, NJT, 3)):
                            eng0 = nc.sync if ci % 2 == 0 else nc.gpsimd
                            eng1 = nc.gpsimd if ci % 2 == 0 else nc.sync
                            eng0.dma_start(
                                out=eb0[:JT, j3 : j3 + 3, :],
                                in_=eb_d[ib, 2 * p, :, j3 : j3 + 3, :],
                            )
                            eng1.dma_start(
                                out=eb1[:JT, j3 : j3 + 3, :],
                                in_=eb_d[ib, 2 * p + 1, :, j3 : j3 + 3, :],
                            )
                        exps = xp.tile([128, NJT, 2, IBW], F16, tag="exps")
                        e = ep.tile([128, NJT, 2, IBW], F16, tag="e")
                        for jt in range(NJT):
                            js = slice(jt * JT, (jt + 1) * JT)
                            s = psS.tile([128, 2, 512], F32, tag="s2")
                            nc.tensor.matmul(
                                s[:JT, 0, :IBW],
                                kt[0:64, p, js],
                                qt[0:64, p, i0 : i0 + IBW],
                                start=True, stop=True,
                                skip_group_check=True,
                                tile_position=(0, 0),
                            )
                            nc.tensor.matmul(
                                s[:JT, 1, :IBW],
                                kt[64:128, p, js],
                                qt[64:128, p, i0 : i0 + IBW],
                                start=True, stop=True,
                                skip_group_check=True,
                                tile_position=(64, 0),
                            )
                            if s1 is not None:
                                pv_acc(s1[0], jt, s1[2], s1[3])
                            nc.scalar.activation(
                                exps[:JT, jt, :, :],
                                s[:JT, :, :IBW],
                                EXPFN,
                                scale=0.125,
                            )
                            if jt % 3 == 2:
                                j0 = jt - 2
                                for hh, ebt in ((0, eb0), (1, eb1)):
                                    nc.vector.tensor_mul(
                                        e[:JT, j0 : jt + 1, hh, :],
                                        exps[:JT, j0 : jt + 1, hh, :],
                                        ebt[:JT, j0 : jt + 1, :],
                                    )
                        if s2 is not None:
                            norm_part2(s2, s2[4])
                            s2 = None
                        if s1 is not None:
                            bc = norm_part1(s1)
                            s2 = (s1[0], s1[1], s1[2], s1[3], bc)
                        pvt = psPV.tile([128, 2, 512], F32, tag="pvt")
                        s1 = (p, ib, pvt, e, None)
                # drain the pipeline
                for jt in range(NJT):
                    pv_acc(s1[0], jt, s1[2], s1[3])
                if s2 is not None:
                    norm_part2(s2, s2[4])
                bc = norm_part1(s1)
                norm_part2((s1[0], s1[1], s1[2], s1[3], bc), bc)
                while proj_queue:
                    pvx = psPV.tile([128, 2, 512], F32, tag="pvt")
                    emit_proj(proj_queue.pop(0), pvx)

    nc.finalize()
    return nc


def _prepare_inputs(x, qkv_w, qkv_b, proj_w, proj_b, rel_pos_table, rel_pos_idx):
    bf = ml_dtypes.bfloat16
    xf = np.asarray(x, np.float32)
    qkv_w = np.asarray(qkv_w, np.float32)
    qkv_b = np.asarray(qkv_b, np.float32)
    proj_w = np.asarray(proj_w, np.float32)
    proj_b = np.asarray(proj_b, np.float32)

    wq = np.ascontiguousarray(
        qkv_w[0:D].reshape(8, 128, 8, 128).transpose(0, 3, 2, 1)
    ).astype(bf)
    wk = np.ascontiguousarray(
        qkv_w[D : 2 * D].reshape(8, 128, 8, 128).transpose(0, 3, 2, 1)
    ).astype(bf)
    wv = np.ascontiguousarray(
        qkv_w[2 * D : 3 * D].reshape(D, 8, 128).transpose(2, 1, 0)
    ).astype(bf)
    wp = np.ascontiguousarray(
        proj_w.reshape(D, 8, 128).transpose(2, 1, 0)
    ).astype(bf)
    qb = np.ascontiguousarray(qkv_b[0:D].reshape(8, 128).T).astype(np.float32)
    kb = np.ascontiguousarray(qkv_b[D : 2 * D].reshape(8, 128).T).astype(np.float32)
    vb = qkv_b[2 * D : 3 * D].reshape(1, D).astype(bf)
    pbr = proj_b.reshape(1, D).astype(bf)

    # exp(bias) factor [ib, h, p, jt, i] f16; pad key j=1025 -> 0
    g = np.asarray(rel_pos_table, np.float32)[np.asarray(rel_pos_idx)]  # [i,j,H]
    full = np.zeros((H, SP, SP), np.float32)                            # [h,j,i]
    full[:, :SEQ, :SEQ] = np.exp(g).transpose(2, 1, 0)
    full[:, :, SEQ:] = 1.0
    full[:, SEQ:, :] = 0.0
    eb = np.ascontiguousarray(
        full.reshape(H, NJT, JT, NIB, IBW).transpose(3, 0, 2, 1, 4)
    ).astype(np.float16)

    in_maps = []
    for b in range(NB):
        xt = np.zeros((128, 8, SP), np.float32)
        xt[:, :, :SEQ] = xf[b].T.reshape(8, 128, SEQ).transpose(1, 0, 2)
        xt = xt.astype(bf)
        in_maps.append(
            {
                "xt": xt, "wq": wq, "wk": wk, "wv": wv, "wp": wp,
                "qb": qb, "kb": kb, "vb": vb, "pb": pbr, "eb": eb,
            }
        )
    return in_maps


def run(inputs, trace=False):
    vbz = not np.any(np.asarray(inputs["qkv_b"], np.float32)[2 * D : 3 * D])
    pbz = not np.any(np.asarray(inputs["proj_b"], np.float32))
    key = ("nc", vbz, pbz)
    if key not in _CACHE:
        _CACHE[key] = _build_module(vbz, pbz)
    nc = _CACHE[key]
    in_maps = _prepare_inputs(**inputs)
    res = run_bass_kernel_spmd(
        nc, in_maps, core_ids=list(range(NB)), trace=trace,
        trace_cores=[0] if trace else None,
    )
    out = np.stack([res.results[b]["y"] for b in range(NB)], axis=0)
    return out, res


def kernel(**inputs) -> np.ndarray:
    out, _ = run(inputs, trace=False)
    return out


# revision 20
# speedup vs baseline: 1.0170x; 1.0170x over previous
"""Trainium2 Bass kernel: BEiT-style dot-product attention with relative
position bias (batch 8, seq 1025, dim 1024, 16 heads), data-parallel over
batch (one batch element per NeuronCore).

v2 design:
  - All GEMM operands bf16 (FWL weight loads, no fp32r self-load stalls).
  - Multiplicative bias: exp(s+b) = exp(s) * exp(b). ScalarE computes
    exp(s*0.125) straight from the QK psum; the host-precomputed f16
    exp(bias) factor is applied by a 2x-rate DVE multiply. No bias matmuls,
    no f32 bias adds, pad key j=1025 masked by exp(bias)=0.
  - seq padded 1025 -> 1026: 9 j-tiles x 114 rows, 3 i-blocks x 342 cols
    (no special width-1 column path).
  - Scores: row-packed QK pairs (2 heads: K=64 rows 0-63 / 64-127).
  - PV: per head stationary [114, 65] = [v | ones]; psum row 64 gives the
    softmax denominator for free.
  - Normalization: reciprocal_approx_fast on the two denominator rows,
    DRAM-roundtrip broadcast to 128 partitions, fused psum-drain multiply.
  - Software pipelining: PV of pair p-1 interleaved with QK of pair p.
  - qkv/proj biases folded into psum drains (per-partition scalar for Q/K,
    broadcast row tiles for V/proj).
"""

import os
import sys

for _p in (
    "/root/.axon_site",
    "/root/.axon_site/_ro/trn_rl_repo",
    "/root/.axon_site/_ro/pypackages",
    "/opt/trn_rl_repo",
    "/opt/pypackages",
):
    if os.path.isdir(_p) and _p not in sys.path:
        sys.path.append(_p)

import numpy as np
import ml_dtypes

import concourse.bass as bass
import concourse.bacc as bacc
import concourse.tile as tile
import concourse.mybir as mybir
from concourse.bass_utils import run_bass_kernel_spmd

F32 = mybir.dt.float32
BF16 = mybir.dt.bfloat16
F16 = mybir.dt.float16
EXPFN = mybir.ActivationFunctionType.Exp

SEQ = 1025
SP = 1026            # padded seq (9 j-tiles of 114; i padded with one zero col)
JT = 114             # j-tile rows
NJT = 9
D = 1024
H = 16
NB = 8
IBW = 342            # i-block width (3 blocks of 342 = 1026)
NIB = 3
QBLK = [(0, 512), (512, 512), (1024, 2)]

_CACHE = {}


def _build_module(vb_zero=False, pb_zero=False):
    nc = bacc.Bacc()
    xt_d = nc.dram_tensor("xt", [128, 8, SP], BF16, kind="ExternalInput")
    wq_d = nc.dram_tensor("wq", [8, 128, 8, 128], BF16, kind="ExternalInput")
    wk_d = nc.dram_tensor("wk", [8, 128, 8, 128], BF16, kind="ExternalInput")
    wv_d = nc.dram_tensor("wv", [128, 8, D], BF16, kind="ExternalInput")
    wp_d = nc.dram_tensor("wp", [128, 8, D], BF16, kind="ExternalInput")
    qb_d = nc.dram_tensor("qb", [128, 8], F32, kind="ExternalInput")
    kb_d = nc.dram_tensor("kb", [128, 8], F32, kind="ExternalInput")
    vb_d = nc.dram_tensor("vb", [1, D], BF16, kind="ExternalInput")
    pb_d = nc.dram_tensor("pb", [1, D], BF16, kind="ExternalInput")
    # exp(bias): [ib, head, p(114), jt(9), i(342)] f16, contiguous per (ib, head)
    eb_d = nc.dram_tensor("eb", [NIB, H, JT, NJT, IBW], F16, kind="ExternalInput")
    y_d = nc.dram_tensor("y", [SEQ, D], F32, kind="ExternalOutput")

    with tile.TileContext(nc) as tc:
        with (
            tc.tile_pool(name="persist", bufs=1) as pp,
            tc.tile_pool(name="consts", bufs=1) as cp,
        ):
            qt = pp.tile([128, 8, SP], BF16, tag="qt")
            kt = pp.tile([128, 8, SP], BF16, tag="kt")
            va = pp.tile([128, NJT, H, 65], F16, tag="va")
            out_all = pp.tile([128, 8, SP], BF16, tag="out_all")
            wp = pp.tile([128, 8, D], BF16, tag="wp")
            for cc2 in range(0, 8, 2):
                nc.sync.dma_start(
                    out=wp[:, cc2 : cc2 + 2, :], in_=wp_d[:, cc2 : cc2 + 2, :],
                    max_dma_last_dim=512,
                )

            qb = cp.tile([128, 8], F32, tag="qb")
            kb = cp.tile([128, 8], F32, tag="kb")
            vb = cp.tile([128, D], BF16, tag="vb")
            pb = cp.tile([128, D], BF16, tag="pb")
            nc.sync.dma_start(out=qb, in_=qb_d[:, :])
            nc.sync.dma_start(out=kb, in_=kb_d[:, :])
            nc.gpsimd.dma_start(
                out=vb,
                in_=bass.AP(tensor=vb_d, offset=0, ap=[[0, 128], [1, D]]),
            )
            nc.gpsimd.dma_start(
                out=pb,
                in_=bass.AP(tensor=pb_d, offset=0, ap=[[0, 128], [1, D]]),
            )
            onesf = cp.tile([128, NJT * H], F32, tag="onesf")
            nc.vector.memset(onesf, 1.0)
            nc.vector.tensor_copy(
                va[:, :, :, 64:65],
                onesf.rearrange("p (t h) -> p t h", t=NJT).unsqueeze(3),
            )

            # ---------------- Phase A: Q, K, V projections ----------------
            with (
                tc.tile_pool(name="xa", bufs=1) as xa,
                tc.tile_pool(name="wl", bufs=3) as wl,
                tc.tile_pool(name="psA", bufs=4, space="PSUM") as psA,
            ):
                xt = xa.tile([128, 8, SP], BF16, tag="xt")
                for ec in range(8):
                    nc.sync.dma_start(out=xt[:, ec, :], in_=xt_d[:, ec, :],
                                       max_dma_last_dim=513)

                for dst, wsrc, bias in ((qt, wq_d, qb), (kt, wk_d, kb)):
                    for ct in range(8):
                        w = wl.tile([128, 8, 128], BF16, tag="wqk")
                        nc.sync.dma_start(out=w, in_=wsrc[ct], max_dma_last_dim=512)
                        for i0, iw in QBLK:
                            pa = psA.tile([128, 512], F32, tag="psA")
                            for ec in range(8):
                                nc.tensor.matmul(
                                    pa[:, :iw],
                                    w[:, ec, :],
                                    xt[:, ec, i0 : i0 + iw],
                                    start=(ec == 0),
                                    stop=(ec == 7),
                                    skip_group_check=True,
                                )
                            nc.scalar.activation(
                                dst[:, ct, i0 : i0 + iw],
                                pa[:, :iw],
                                mybir.ActivationFunctionType.Identity,
                                bias=bias[:, ct : ct + 1],
                            )

                wv = xa.tile([128, 8, D], BF16, tag="wv")
                for ec2 in range(0, 8, 2):
                    nc.sync.dma_start(
                        out=wv[:, ec2 : ec2 + 2, :], in_=wv_d[:, ec2 : ec2 + 2, :],
                        max_dma_last_dim=512,
                    )
                for jt in range(NJT):
                    js = slice(jt * JT, (jt + 1) * JT)
                    for hb in range(2):
                        ms = slice(hb * 512, (hb + 1) * 512)
                        pa = psA.tile([128, 512], F32, tag="psA")
                        for ec in range(8):
                            nc.tensor.matmul(
                                pa[:JT, :],
                                xt[:, ec, js],
                                wv[:, ec, ms],
                                start=(ec == 0),
                                stop=(ec == 7),
                                skip_group_check=True,
                            )
                        if vb_zero:
                            nc.scalar.copy(
                                va[:JT, jt, hb * 8 : (hb + 1) * 8, 0:64],
                                pa[:JT, :].rearrange("p (h c) -> p h c", c=64),
                            )
                        else:
                            nc.vector.tensor_add(
                                va[:JT, jt, hb * 8 : (hb + 1) * 8, 0:64],
                                pa[:JT, :].rearrange("p (h c) -> p h c", c=64),
                                vb[:JT, ms].rearrange("p (h c) -> p h c", c=64),
                            )

            # ---------------- Phase B: attention ----------------
            # 3-deep software pipeline over pairs i (= ib*8+p):
            #   block i emits: eb-dma(i+..), QK(i,jt)+PV(i-1,jt)+exp(i,jt)
            #   + mult chunks(i), norm-part1(i-1) {dn,recip,cast,rd,bc},
            #   norm-part2(i-2) {muls -> out_all} + proj chunk when its ib done.
            with (
                tc.tile_pool(name="psS", bufs=2, space="PSUM") as psS,
                tc.tile_pool(name="psPV", bufs=2, space="PSUM") as psPV,
                tc.tile_pool(name="ebp", bufs=3) as ebp,
                tc.tile_pool(name="exps", bufs=2) as xp,
                tc.tile_pool(name="rp", bufs=3) as rp,
                tc.tile_pool(name="dramp", bufs=3, space="DRAM") as dp,
                tc.tile_pool(name="yp", bufs=2) as yp,
            ):
                def emit_proj(mt, pvt):
                    i0 = 128 * mt if mt < 8 else SEQ - 128
                    ysb = yp.tile([128, D], F32, tag="ysb")
                    for fb in range(2):
                        fs = slice(fb * 512, (fb + 1) * 512)
                        pj = pvt[:, fb, :]
                        for cc in range(8):
                            nc.tensor.matmul(
                                pj,
                                out_all[:, cc, i0 : i0 + 128],
                                wp[:, cc, fs],
                                start=(cc == 0),
                                stop=(cc == 7),
                                skip_group_check=True,
                            )
                        if pb_zero:
                            if fb == 0:
                                nc.scalar.copy(ysb[:, fs], pj)
                            else:
                                nc.vector.tensor_copy(ysb[:, fs], pj)
                        else:
                            nc.vector.tensor_add(ysb[:, fs], pj, pb[:, fs])
                    if mt < 8:
                        for yh in range(2):
                            nc.scalar.dma_start(
                                out=y_d[i0 + 64 * yh : i0 + 64 * (yh + 1), :],
                                in_=ysb[64 * yh : 64 * (yh + 1), :],
                            )
                    else:
                        nc.scalar.dma_start(
                            out=y_d[SEQ - 1 : SEQ, :], in_=ysb[127:128, :]
                        )

                def pv_acc(p, jt, pvt, e):
                    for hh in range(2):
                        nc.tensor.matmul(
                            pvt[0:65, hh, :IBW],
                            va[:JT, jt, 2 * p + hh, :],
                            e[:JT, jt, hh, :],
                            start=(jt == 0),
                            stop=(jt == NJT - 1),
                            skip_group_check=True,
                        )

                def norm_part1(st):
                    # denominators -> approx reciprocal -> bf16 -> DRAM
                    p, ib, pvt, _, _ = st
                    dn = rp.tile([1, 2, IBW], F32, tag="dn")
                    nc.vector.tensor_copy(dn, pvt[64:65, :, :IBW])
                    rr = rp.tile([1, 2, IBW], F32, tag="rr")
                    nc.vector.reciprocal_approx_fast(out=rr, in_=dn)
                    rb = rp.tile([1, 2, IBW], BF16, tag="rb")
                    nc.vector.tensor_copy(rb, rr)
                    rd = dp.tile([2, IBW], BF16, tag="rd")
                    nc.sync.dma_start(out=rd, in_=rb[0:1, :, :])
                    bc = rp.tile([128, IBW], BF16, tag="bc")
                    nc.gpsimd.dma_start(
                        out=bc,
                        in_=bass.AP(
                            tensor=rd.tensor, offset=rd.offset,
                            ap=[[IBW, 2], [0, 64], [1, IBW]],
                        ),
                    )
                    return bc

                def norm_part2(st, bc):
                    p, ib, pvt, _, _ = st
                    i0 = ib * IBW
                    tmp = rp.tile([128, IBW], F32, tag="tmp")
                    nc.vector.tensor_mul(
                        out_all[0:64, p, i0 : i0 + IBW],
                        pvt[0:64, 0, :IBW],
                        bc[0:64, :],
                    )
                    nc.vector.tensor_copy(tmp[64:128, :], pvt[0:64, 1, :IBW])
                    nc.vector.tensor_mul(
                        out_all[64:128, p, i0 : i0 + IBW],
                        tmp[64:128, :],
                        bc[64:128, :],
                    )
                    if proj_queue:
                        emit_proj(proj_queue.pop(0), pvt)
                    if p == 7:
                        proj_queue.extend(PROJ_SETS[ib])

                PROJ_SETS = {0: [0, 1], 1: [2, 3, 4], 2: [5, 6, 7, 8]}
                proj_queue = []
                s1 = None   # (p, ib, pvt, e, bc-pending)
                s2 = None
                for ib in range(NIB):
                    i0 = ib * IBW
                    for p in range(8):
                        eb0 = ebp.tile([128, NJT, IBW], F16, tag="eb0")
                        eb1 = ebp.tile([128, NJT, IBW], F16, tag="eb1")
                        for ci, j3 in enumerate(range(0, NJT, 3)):
                            eng0 = nc.sync if ci % 2 == 0 else nc.gpsimd
                            eng1 = nc.gpsimd if ci % 2 == 0 else nc.sync
                            eng0.dma_start(
                                out=eb0[:JT, j3 : j3 + 3, :],
                                in_=eb_d[ib, 2 * p, :, j3 : j3 + 3, :],
                            )
                            eng1.dma_start(
                                out=eb1[:JT, j3 : j3 + 3, :],
                                in_=eb_d[ib, 2 * p + 1, :, j3 : j3 + 3, :],
                            )
                        exps = xp.tile([128, NJT, 2, IBW], F16, tag="exps")
                        e = exps
                        for jt in range(NJT):
                            js = slice(jt * JT, (jt + 1) * JT)
                            s = psS.tile([128, 2, 512], F32, tag="s2")
                            nc.tensor.matmul(
                                s[:JT, 0, :IBW],
                                kt[0:64, p, js],
                                qt[0:64, p, i0 : i0 + IBW],
                                start=True, stop=True,
                                skip_group_check=True,
                                tile_position=(0, 0),
                            )
                            nc.tensor.matmul(
                                s[:JT, 1, :IBW],
                                kt[64:128, p, js],
                                qt[64:128, p, i0 : i0 + IBW],
                                start=True, stop=True,
                                skip_group_check=True,
                                tile_position=(64, 0),
                            )
                            if s1 is not None:
                                pv_acc(s1[0], jt, s1[2], s1[3])
                            nc.scalar.activation(
                                exps[:JT, jt, :, :],
                                s[:JT, :, :IBW],
                                EXPFN,
                                scale=0.125,
                            )
                            if jt % 3 == 2:
                                j0 = jt - 2
                                for hh, ebt in ((0, eb0), (1, eb1)):
                                    nc.vector.tensor_mul(
                                        e[:JT, j0 : jt + 1, hh, :],
                                        exps[:JT, j0 : jt + 1, hh, :],
                                        ebt[:JT, j0 : jt + 1, :],
                                    )
                        if s2 is not None:
                            norm_part2(s2, s2[4])
                            s2 = None
                        if s1 is not None:
                            bc = norm_part1(s1)
                            s2 = (s1[0], s1[1], s1[2], s1[3], bc)
                        pvt = psPV.tile([128, 2, 512], F32, tag="pvt")
                        s1 = (p, ib, pvt, e, None)
                # drain the pipeline
                for jt in range(NJT):
                    pv_acc(s1[0], jt, s1[2], s1[3])
                if s2 is not None:
                    norm_part2(s2, s2[4])
                bc = norm_part1(s1)
                norm_part2((s1[0], s1[1], s1[2], s1[3], bc), bc)
                while proj_queue:
                    pvx = psPV.tile([128, 2, 512], F32, tag="pvt")
                    emit_proj(proj_queue.pop(0), pvx)

    nc.finalize()
    return nc


def _prepare_inputs(x, qkv_w, qkv_b, proj_w, proj_b, rel_pos_table, rel_pos_idx):
    bf = ml_dtypes.bfloat16
    xf = np.asarray(x, np.float32)
    qkv_w = np.asarray(qkv_w, np.float32)
    qkv_b = np.asarray(qkv_b, np.float32)
    proj_w = np.asarray(proj_w, np.float32)
    proj_b = np.asarray(proj_b, np.float32)

    wq = np.ascontiguousarray(
        qkv_w[0:D].reshape(8, 128, 8, 128).transpose(0, 3, 2, 1)
    ).astype(bf)
    wk = np.ascontiguousarray(
        qkv_w[D : 2 * D].reshape(8, 128, 8, 128).transpose(0, 3, 2, 1)
    ).astype(bf)
    wv = np.ascontiguousarray(
        qkv_w[2 * D : 3 * D].reshape(D, 8, 128).transpose(2, 1, 0)
    ).astype(bf)
    wp = np.ascontiguousarray(
        proj_w.reshape(D, 8, 128).transpose(2, 1, 0)
    ).astype(bf)
    qb = np.ascontiguousarray(qkv_b[0:D].reshape(8, 128).T).astype(np.float32)
    kb = np.ascontiguousarray(qkv_b[D : 2 * D].reshape(8, 128).T).astype(np.float32)
    vb = qkv_b[2 * D : 3 * D].reshape(1, D).astype(bf)
    pbr = proj_b.reshape(1, D).astype(bf)

    # exp(bias) factor [ib, h, p, jt, i] f16; pad key j=1025 -> 0
    g = np.asarray(rel_pos_table, np.float32)[np.asarray(rel_pos_idx)]  # [i,j,H]
    full = np.zeros((H, SP, SP), np.float32)                            # [h,j,i]
    full[:, :SEQ, :SEQ] = np.exp(g).transpose(2, 1, 0)
    full[:, :, SEQ:] = 1.0
    full[:, SEQ:, :] = 0.0
    eb = np.ascontiguousarray(
        full.reshape(H, NJT, JT, NIB, IBW).transpose(3, 0, 2, 1, 4)
    ).astype(np.float16)

    in_maps = []
    for b in range(NB):
        xt = np.zeros((128, 8, SP), np.float32)
        xt[:, :, :SEQ] = xf[b].T.reshape(8, 128, SEQ).transpose(1, 0, 2)
        xt = xt.astype(bf)
        in_maps.append(
            {
                "xt": xt, "wq": wq, "wk": wk, "wv": wv, "wp": wp,
                "qb": qb, "kb": kb, "vb": vb, "pb": pbr, "eb": eb,
            }
        )
    return in_maps


def run(inputs, trace=False):
    vbz = not np.any(np.asarray(inputs["qkv_b"], np.float32)[2 * D : 3 * D])
    pbz = not np.any(np.asarray(inputs["proj_b"], np.float32))
    key = ("nc", vbz, pbz)
    if key not in _CACHE:
        _CACHE[key] = _build_module(vbz, pbz)
    nc = _CACHE[key]
    in_maps = _prepare_inputs(**inputs)
    res = run_bass_kernel_spmd(
        nc, in_maps, core_ids=list(range(NB)), trace=trace,
        trace_cores=[0] if trace else None,
    )
    out = np.stack([res.results[b]["y"] for b in range(NB)], axis=0)
    return out, res


def kernel(**inputs) -> np.ndarray:
    out, _ = run(inputs, trace=False)
    return out
